# revision 35
# baseline (speedup 1.0000x reference)
"""Trainium2 Bass kernel for nn_DatastoreReaderLayer (retrieval kNN attention).

Strategy (8 NeuronCores, datastore sharded over N):
  - Each core owns an N/8 = 4096-row shard of the datastore.
  - K/V weight projections are algebraically absorbed:
      logits = q @ Wq.T @ Wk @ dstore_k.T   (Wqk := Wq.T @ Wk folded on host,
                                             softmax scale folded into Wqk)
      attn   = (softmax @ dstore_v) @ Wv.T  (wv := e @ dstore_v on device)
    so the two big [N,512]x[512,512] projection matmuls vanish.
  - All large matmuls run in bf16 (same 1 cycle/row issue rate as fp32r on
    TRN2, but LDWEIGHTS halves and HBM traffic halves); accumulation is fp32
    in PSUM.  dv is pre-packed on the host into partition-major layout so its
    stream DMAs are long contiguous lines.
  - Softmax without max-subtraction (logits ~ N(0,0.6), exp is safe in fp32);
    partial sum-exp and partial unnormalized attn are combined across cores
    with a single bf16 ReduceScatter per query half (flash-attention merge).
  - Each core finishes the gate MLP for its own 128 query rows and returns a
    [128, 512] slice; the host reassembles the full [256, 4, 512] output.
  - bk is provably a softmax no-op; bq folds into a qk bias vector; bv/bg1/bg2
    are applied exactly on device.
"""

import sys

for _p in ("/opt/trn_rl_repo", "/root/.axon_site/_ro/trn_rl_repo"):
    if _p not in sys.path:
        sys.path.append(_p)

import numpy as np
import ml_dtypes

import concourse.tile as tile
from concourse import bacc, bass_isa, mybir
from concourse.bass_utils import run_bass_kernel_spmd

SEQ, BATCH, D, NTOT = 256, 4, 512, 32768
TEMP = 0.5
NCORES = 8
NS = NTOT // NCORES  # datastore rows per core
SB = SEQ * BATCH  # 1024 query rows, b-major (row r = b*SEQ + s)
F32 = mybir.dt.float32
BF16 = mybir.dt.bfloat16
AF = mybir.ActivationFunctionType
ALU = mybir.AluOpType
BF = ml_dtypes.bfloat16

_PROGRAM_CACHE: dict = {}


def build_program(ns: int, bg2f: float, reps: int = 1):
    """One SPMD program; per-core data differences come via in_maps.

    reps > 1 statically repeats the whole computation (for wall-clock-delta
    timing of the kernel proper); the output is written identically each rep.
    """
    nchunks = ns // 128
    nc = bacc.Bacc(None, target_bir_lowering=False, debug=False, num_devices=NCORES)

    def inp(nm, shp, dt=F32):
        return nc.declare_dram_parameter(nm, list(shp), dt, isOutput=False)

    qkT_d = inp("qkT", (D, SB), BF16)        # (qb @ Wqk + bias)^T, host-computed
    dkT_d = inp("dkT", (D, ns), BF16)        # datastore-K shard, transposed
    dvp_d = inp("dvp", (128, (ns // 128) * D), BF16)  # V shard, partition-packed
    wvT_d = inp("wvT", (D, D), BF16)         # Wv.T
    wg1T_d = inp("wg1T", (2 * D, D), BF16)   # Wg1.T
    wg2r_d = inp("wg2r", (128, D))           # Wg2 replicated over partitions
    bvr_d = inp("bvr", (128, D))
    bg1r_d = inp("bg1r", (128, D))
    prevN_d = inp("prevN", (128, D))         # prev rows for this core's slice
    prevT_d = inp("prevT", (D, 128), BF16)   # same rows, transposed
    ident_d = inp("ident", (128, 128))
    out_d = nc.declare_dram_parameter("out", [128, D], F32, isOutput=True)

    rg = [list(range(NCORES))]

    def emit_body(nc, tc, pools, rp):
        cp, sp, xp, wgp, mmp, wvp, dp = pools

        def cload(src_ap, shape, tg, dt=F32):
            t = cp.tile(shape, dt, tag=tg, name=rp + tg)
            nc.sync.dma_start(t[:], src_ap)
            return t

        # q-side loads go first in the sync DMA FIFO so the first logits
        # chunk can compute while the bulk dkT load streams in behind it;
        # the first pieces are small so chunk 0 is unblocked early.
        qkT = [cp.tile([128, SB], BF16, tag=f"qkT{k}", name=rp + f"qkT{k}")
               for k in range(4)]
        for k in range(4):
            nc.sync.dma_start(qkT[k][:, 0:512], qkT_d[k * 128:(k + 1) * 128, 0:512])
        dkT = [cp.tile([128, ns], BF16, tag=f"dkT{k}", name=rp + f"dkT{k}")
               for k in range(4)]
        for k in range(4):
            nc.sync.dma_start(dkT[k][:, 0:256], dkT_d[k * 128:(k + 1) * 128, 0:256])
        for k in range(4):
            nc.sync.dma_start(qkT[k][:, 512:SB],
                              qkT_d[k * 128:(k + 1) * 128, 512:SB])
        for o in range(256, ns, 1024):
            o2 = min(o + 1024, ns)
            for k in range(4):
                nc.sync.dma_start(dkT[k][:, o:o2],
                                  dkT_d[k * 128:(k + 1) * 128, o:o2])
        wvT = [cload(wvT_d[k * 128:(k + 1) * 128, :], [128, D], f"wvT{k}", BF16)
               for k in range(4)]
        wg2r = cload(wg2r_d[:], [128, D], "wg2r")
        bvr = cload(bvr_d[:], [128, D], "bvr")
        bg1r = cload(bg1r_d[:], [128, D], "bg1r")
        prevN = cload(prevN_d[:], [128, D], "prevN")
        prevT = [cload(prevT_d[k * 128:(k + 1) * 128, :], [128, 128],
                       f"prevT{k}", BF16) for k in range(4)]
        ident = cload(ident_d[:], [128, 128], "ident")
        ones = cp.tile([128, 1], F32, tag="ones", name=rp + "ones")
        nc.vector.memset(ones[:], 1.0)
        ones_b = cp.tile([128, 1], BF16, tag="ones_b", name=rp + "ones_b")
        nc.vector.memset(ones_b[:], 1.0)
        ident_b = cp.tile([128, 128], BF16, tag="ident_b", name=rp + "ident_b")
        nc.vector.tensor_copy(ident_b[:], ident[:])

        # dv resident in SBUF (bf16, 4 MB): loaded once on the ACT-issued
        # queue in column pieces, reused by both query halves.
        dvt = cp.tile([128, nchunks * D], BF16, tag="dvt", name=rp + "dvt")
        for j in range(0, nchunks, 4):
            nc.scalar.dma_start(dvt[:, j * D:(j + 4) * D],
                                dvp_d[:, j * D:(j + 4) * D])

        # exp(logits) for ALL queries, kept resident (bf16, 8 MB): written by
        # the h=0 pass at full 1024-wide matmuls, re-read by the h=1 AV pass
        # and the sum-exp matmuls.
        e_st = cp.tile([128, nchunks * SB], BF16, tag="e_st", name=rp + "e_st")

        wvacc = [cp.tile([128, SB], BF16, tag=f"wvacc{k}", name=rp + f"wvacc{k}")
                 for k in range(4)]

        cc_in = [dp.tile([SB // 2, 513], BF16, tag=f"ccin{h}",
                         name=rp + f"ccin{h}") for h in range(2)]
        cc_out = [dp.tile([SB // 16, 513], BF16, tag=f"ccout{h}",
                          name=rp + f"ccout{h}") for h in range(2)]

        def sum_exp(h):
            # sum-exp over the datastore shard for query half h, entirely off
            # the tensor engine: strided chunk-reduction on DVE in 8 pieces
            # (all but the last run during the main loop), running-sum
            # accumulation, then a partition-reduction on GpSimd.  Row 0 of
            # the returned broadcast tile is the sum-exp row.
            qs = 512 * h
            ev = e_st[:].rearrange("p (j q) -> p j q", j=nchunks)
            npc = nchunks // 8
            acc = sp.tile([128, 512], F32, tag="scr", name=rp + f"seacc{h}")
            for pi in range(8):
                src = ev[:, pi * npc:(pi + 1) * npc, qs:qs + 512] \
                    .transpose([0, 2, 1])
                if pi == 0:
                    nc.vector.tensor_reduce(acc[:], src,
                                            axis=mybir.AxisListType.X, op=ALU.add)
                else:
                    ep_t = sp.tile([128, 512], F32, tag="scr",
                                   name=rp + f"sep{h}{pi}")
                    nc.vector.tensor_reduce(ep_t[:], src,
                                            axis=mybir.AxisListType.X, op=ALU.add)
                    nc.vector.tensor_tensor(acc[:], acc[:], ep_t[:], op=ALU.add)
            spar = cp.tile([128, 512], F32, tag=f"spar{h}", name=rp + f"spar{h}")
            nc.gpsimd.partition_all_reduce(spar[:], acc[:], channels=128,
                                           reduce_op=bass_isa.ReduceOp.add)
            return spar

        def pa_rs(h, spar):
            # unnormalized attn slices + sum-exp column -> cc_in, then RS.
            # pa matmuls first (they don't need the sum-exp, which may still
            # be finishing on DVE/GpSimd), then psc + payload assembly.
            for jj in range(4):
                g = h * 4 + jj
                pa = mmp.tile([128, 512], F32, tag="mm", name=rp + f"pa{g}")
                for k in range(4):
                    nc.tensor.matmul(
                        pa[:], wvacc[k][:, g * 128:(g + 1) * 128],
                        wvT[k][:], start=(k == 0), stop=(k == 3))
                ext = xp.tile([128, 513], BF16, tag="ext", name=rp + f"ext{g}")
                nc.vector.tensor_copy(ext[:, 0:512], pa[:])
                psc = wvp.tile([128, 1], F32, tag="wv", name=rp + f"psc{g}")
                nc.tensor.matmul(psc[:], spar[0:1, jj * 128:(jj + 1) * 128],
                                 ones[0:1, 0:1], start=True, stop=True)
                nc.vector.tensor_copy(ext[:, 512:513], psc[:])
                nc.sync.dma_start(cc_in[h][jj * 128:(jj + 1) * 128, :], ext[:])
            nc.gpsimd.collective_compute(
                "ReduceScatter", ALU.add, replica_groups=rg,
                ins=[cc_in[h].opt()], outs=[cc_out[h].opt()])

        # ---- pass 1: logits for BOTH query halves (paired on the shared dkT
        # stationary), exp into e_st, AV accumulation for half 0.  The next
        # chunk's logits are emitted ahead of this chunk's AV so the tensor
        # queue never stalls on the ACT exp dependency.
        wv_ps = [wvp.tile([128, 512], F32, tag="wv", name=rp + f"wv0{k}")
                 for k in range(4)]

        def logits(j):
            pl0 = mmp.tile([128, 512], F32, tag="mm", name=rp + f"pl{j}a")
            pl1 = mmp.tile([128, 512], F32, tag="mm", name=rp + f"pl{j}b")
            for k in range(4):
                nc.tensor.matmul(
                    pl0[:], dkT[k][:, j * 128:(j + 1) * 128],
                    qkT[k][:, 0:512], start=(k == 0), stop=(k == 3))
                nc.tensor.matmul(
                    pl1[:], dkT[k][:, j * 128:(j + 1) * 128],
                    qkT[k][:, 512:SB], start=(k == 0), stop=(k == 3))
            nc.scalar.activation(e_st[:, j * SB:j * SB + 512], pl0[:], AF.Exp)
            nc.scalar.activation(e_st[:, j * SB + 512:(j + 1) * SB], pl1[:],
                                 AF.Exp)

        logits(0)
        for j in range(nchunks):
            if j + 1 < nchunks:
                logits(j + 1)
            for k in range(4):
                nc.tensor.matmul(
                    wv_ps[k][:],
                    dvt[:, j * D + k * 128:j * D + (k + 1) * 128],
                    e_st[:, j * SB:j * SB + 512],
                    start=(j == 0), stop=(j == nchunks - 1))
        spar0 = sum_exp(0)
        for k in range(4):
            nc.vector.tensor_copy(wvacc[k][:, 0:512], wv_ps[k][:])
        pa_rs(0, spar0)

        # ---- pass 2: AV + sum-exp for half 1 from stored e (no deps at all;
        # the sum-exp matmuls ride the tensor engine here since pass 2 runs
        # concurrently with the half-0 ReduceScatter anyway, and this keeps
        # the DVE queue clear for the RS1 payload assembly)
        wv_ps = [wvp.tile([128, 512], F32, tag="wv", name=rp + f"wv1{k}")
                 for k in range(4)]
        se_ps = mmp.tile([1, 512], F32, tag="mm", name=rp + "se1")
        for j in range(nchunks):
            for k in range(4):
                nc.tensor.matmul(
                    wv_ps[k][:],
                    dvt[:, j * D + k * 128:j * D + (k + 1) * 128],
                    e_st[:, j * SB + 512:(j + 1) * SB],
                    start=(j == 0), stop=(j == nchunks - 1))
            nc.tensor.matmul(se_ps[:], ones_b[:],
                             e_st[:, j * SB + 512:(j + 1) * SB],
                             start=(j == 0), stop=(j == nchunks - 1))
        srow1 = cp.tile([1, 512], F32, tag="srow1", name=rp + "srow1")
        nc.vector.tensor_copy(srow1[:], se_ps[:])
        for k in range(4):
            nc.vector.tensor_copy(wvacc[k][:, 512:SB], wv_ps[k][:])
        pa_rs(1, srow1)

        # ---- post-RS: this core's 128 query rows.
        # wg1T streams first (no deps -> overlaps the RS wait) and the
        # prev-side half of the gate matmul is accumulated during the wait.
        wg1T = [None] * 8
        for k in list(range(4, 8)) + list(range(4)):
            t = wgp.tile([128, D], BF16, tag="wg", name=rp + f"wg1T{k}")
            nc.sync.dma_start(t[:], wg1T_d[k * 128:(k + 1) * 128, :])
            wg1T[k] = t
        ph2 = mmp.tile([128, D], F32, tag="mm", name=rp + "ph2")
        for k in range(4):
            nc.tensor.matmul(ph2[:], prevT[k][:], wg1T[4 + k][:],
                             start=(k == 0), stop=(k == 3))
        hprev = sp.tile([128, D], F32, tag="scr", name=rp + "hprev")
        nc.vector.tensor_tensor(hprev[:], ph2[:], bg1r[:], op=ALU.add)

        post = cp.tile([128, 513], BF16, tag="post", name=rp + "post")
        nc.sync.dma_start(post[0:64, :], cc_out[0][:])
        nc.sync.dma_start(post[64:128, :], cc_out[1][:])
        a32 = sp.tile([128, D], F32, tag="scr", name=rp + "a32")
        nc.vector.tensor_copy(a32[:], post[:, 0:512])
        s32 = cp.tile([128, 1], F32, tag="s32", name=rp + "s32")
        nc.vector.tensor_copy(s32[:], post[:, 512:513])
        recip = cp.tile([128, 1], F32, tag="recip", name=rp + "recip")
        nc.vector.reciprocal(recip[:], s32[:])
        attn = sp.tile([128, D], F32, tag="scr", name=rp + "attn")
        nc.vector.scalar_tensor_tensor(
            attn[:], a32[:], recip[:], bvr[:],
            op0=ALU.mult, op1=ALU.add)
        attn_b = cp.tile([128, D], BF16, tag="attn_b", name=rp + "attn_b")
        nc.vector.tensor_copy(attn_b[:], attn[:])

        aTall = cp.tile([128, D], BF16, tag="aTall", name=rp + "aTall")
        for k in range(4):
            pt = mmp.tile([128, 128], BF16, tag="mm", name=rp + f"pt{k}")
            nc.tensor.transpose(pt[:], attn_b[:, k * 128:(k + 1) * 128],
                                ident_b[:])
            nc.vector.tensor_copy(aTall[:, k * 128:(k + 1) * 128], pt[:])

        ph = mmp.tile([128, D], F32, tag="mm", name=rp + "ph")
        for k in range(4):
            nc.tensor.matmul(ph[:], aTall[:, k * 128:(k + 1) * 128],
                             wg1T[k][:], start=(k == 0), stop=(k == 3))
        hsum = sp.tile([128, D], F32, tag="scr", name=rp + "hsum")
        nc.vector.tensor_tensor(hsum[:], ph[:], hprev[:], op=ALU.add)
        hrelu = sp.tile([128, D], F32, tag="scr", name=rp + "hrelu")
        nc.scalar.activation(hrelu[:], hsum[:], AF.Relu)

        tmp = sp.tile([128, D], F32, tag="scr", name=rp + "tmp")
        sigp = cp.tile([128, 1], F32, tag="sigp", name=rp + "sigp")
        nc.vector.scalar_tensor_tensor(
            tmp[:], hrelu[:], 1.0, wg2r[:],
            op0=ALU.mult, op1=ALU.mult, accum_out=sigp[:])
        # sigma = 0.5 + 0.5*tanh(0.5*(x + bg2)); tanh shares the Exp table set
        tnh = cp.tile([128, 1], F32, tag="tnh", name=rp + "tnh")
        nc.scalar.activation(tnh[:], sigp[:], AF.Tanh,
                             scale=0.5, bias=0.5 * bg2f)

        dlt = sp.tile([128, D], F32, tag="scr", name=rp + "dlt")
        nc.vector.tensor_tensor(dlt[:], attn[:], prevN[:], op=ALU.subtract)
        x1 = sp.tile([128, D], F32, tag="scr", name=rp + "x1")
        nc.vector.tensor_scalar_mul(x1[:], dlt[:], tnh[:])
        z = sp.tile([128, D], F32, tag="scr", name=rp + "z")
        nc.vector.tensor_tensor(z[:], dlt[:], x1[:], op=ALU.add)
        res = sp.tile([128, D], F32, tag="scr", name=rp + "res")
        nc.vector.scalar_tensor_tensor(
            res[:], z[:], 0.5, prevN[:], op0=ALU.mult, op1=ALU.add)
        nc.sync.dma_start(out_d[:], res[:])

    with tile.TileContext(nc) as tc:
        with (
            tc.tile_pool(name="const", bufs=1) as cp,
            tc.tile_pool(name="scratch", bufs=8) as sp,
            tc.tile_pool(name="xp", bufs=2) as xp,
            tc.tile_pool(name="wgp", bufs=8) as wgp,
            tc.tile_pool(name="mm", bufs=4, space="PSUM") as mmp,
            tc.tile_pool(name="wvp", bufs=4, space="PSUM") as wvp,
            tc.tile_pool(name="dram", bufs=1, space="DRAM") as dp,
        ):
            pools = (cp, sp, xp, wgp, mmp, wvp, dp)
            for rep in range(reps):
                emit_body(nc, tc, pools, f"r{rep}_" if reps > 1 else "")

    nc.finalize()
    return nc


def make_in_maps(q, prev, Wq, bq, Wk, Wv, Wg1, Wg2, bg2, bv, bg1,
                 dstore_k, dstore_v, ns):
    """Host-side sharding + layout prep. Returns per-core input dicts."""
    alpha = (D ** -0.5) / TEMP
    f = np.float32
    qb = np.ascontiguousarray(q.transpose(1, 0, 2).reshape(SB, D), dtype=f)
    prevb = np.ascontiguousarray(prev.transpose(1, 0, 2).reshape(SB, D), dtype=f)
    wqk = (Wq.T.astype(np.float64) @ Wk.astype(np.float64) * alpha).astype(f)
    qkb = ((bq.astype(np.float64) @ Wk.astype(np.float64)) * alpha).astype(f)
    qkT = np.ascontiguousarray((qb @ wqk + qkb).T.astype(BF))  # [D, SB]
    wvT = np.ascontiguousarray(Wv.T.astype(BF))
    wg1T = np.ascontiguousarray(Wg1.T.astype(BF))
    wg2r = np.ascontiguousarray(np.broadcast_to(Wg2.reshape(1, D), (128, D)), dtype=f)
    bvr = np.ascontiguousarray(np.broadcast_to(bv.reshape(1, D), (128, D)), dtype=f)
    bg1r = np.ascontiguousarray(np.broadcast_to(bg1.reshape(1, D), (128, D)), dtype=f)
    ident = np.eye(128, dtype=f)
    half = SB // 2
    sl = half // NCORES  # 64 rows per half per core
    nch = ns // 128

    dkT_all = np.ascontiguousarray(dstore_k.T.astype(BF))      # [D, NTOT]
    dv_bf = dstore_v.astype(BF)                                # [NTOT, D]

    in_maps = []
    for c in range(NCORES):
        rows = np.r_[c * sl:(c + 1) * sl, half + c * sl:half + (c + 1) * sl]
        prevN = np.ascontiguousarray(prevb[rows])
        prevT = np.ascontiguousarray(prevN.T.astype(BF))
        dkT = np.ascontiguousarray(dkT_all[:, c * ns:(c + 1) * ns])
        # partition-major packing: dvp[p, j*D + d] = dv[c*ns + j*128 + p, d]
        dvp = np.ascontiguousarray(
            dv_bf[c * ns:(c + 1) * ns].reshape(nch, 128, D)
            .transpose(1, 0, 2).reshape(128, nch * D))
        in_maps.append({
            "qkT": qkT, "dkT": dkT, "dvp": dvp,
            "wvT": wvT, "wg1T": wg1T, "wg2r": wg2r, "bvr": bvr, "bg1r": bg1r,
            "prevN": prevN, "prevT": prevT, "ident": ident,
        })
    return in_maps


def assemble_output(core_outs):
    """[128,512] per core -> [SEQ, BATCH, D] full output."""
    half = SB // 2
    sl = half // NCORES
    res_bm = np.empty((SB, D), dtype=np.float32)
    for c in range(NCORES):
        res_bm[c * sl:(c + 1) * sl] = core_outs[c][0:sl]
        res_bm[half + c * sl:half + (c + 1) * sl] = core_outs[c][sl:2 * sl]
    return np.ascontiguousarray(
        res_bm.reshape(BATCH, SEQ, D).transpose(1, 0, 2))


def kernel(q, prev_layer_output, Wq, bq, Wk, bk, Wv, bv, Wg1, bg1, Wg2, bg2,
           dstore_k, dstore_v):
    # bk shifts every logit in a row by a constant -> softmax-invariant; unused.
    ns = NTOT // NCORES
    bg2f = float(np.asarray(bg2).reshape(-1)[0])
    key = (ns, bg2f, 1)
    if key not in _PROGRAM_CACHE:
        _PROGRAM_CACHE[key] = build_program(ns, bg2f)
    nc = _PROGRAM_CACHE[key]
    in_maps = make_in_maps(q, prev_layer_output, Wq, bq, Wk, Wv, Wg1, Wg2, bg2,
                           bv, bg1, dstore_k, dstore_v, ns)
    res = run_bass_kernel_spmd(nc, in_maps, list(range(NCORES)))
    return assemble_output([res.results[c]["out"] for c in range(NCORES)])


# revision 37
# speedup vs baseline: 1.0947x; 1.0947x over previous
"""Trainium2 Bass kernel for nn_DatastoreReaderLayer (retrieval kNN attention).

Strategy (8 NeuronCores, datastore sharded over N):
  - Each core owns an N/8 = 4096-row shard of the datastore.
  - K/V weight projections are algebraically absorbed:
      logits = q @ Wq.T @ Wk @ dstore_k.T   (Wqk := Wq.T @ Wk folded on host,
                                             softmax scale folded into Wqk)
      attn   = (softmax @ dstore_v) @ Wv.T  (wv := e @ dstore_v on device)
    so the two big [N,512]x[512,512] projection matmuls vanish.
  - All large matmuls run in bf16 (same 1 cycle/row issue rate as fp32r on
    TRN2, but LDWEIGHTS halves and HBM traffic halves); accumulation is fp32
    in PSUM.  dv is pre-packed on the host into partition-major layout so its
    stream DMAs are long contiguous lines.
  - Softmax without max-subtraction (logits ~ N(0,0.6), exp is safe in fp32);
    partial sum-exp and partial unnormalized attn are combined across cores
    with a single bf16 ReduceScatter per query half (flash-attention merge).
  - Each core finishes the gate MLP for its own 128 query rows and returns a
    [128, 512] slice; the host reassembles the full [256, 4, 512] output.
  - bk is provably a softmax no-op; bq folds into a qk bias vector; bv/bg1/bg2
    are applied exactly on device.
"""

import sys

for _p in ("/opt/trn_rl_repo", "/root/.axon_site/_ro/trn_rl_repo"):
    if _p not in sys.path:
        sys.path.append(_p)

import numpy as np
import ml_dtypes

import concourse.tile as tile
from concourse import bacc, bass_isa, mybir
from concourse.bass_utils import run_bass_kernel_spmd

SEQ, BATCH, D, NTOT = 256, 4, 512, 32768
TEMP = 0.5
NCORES = 8
NS = NTOT // NCORES  # datastore rows per core
SB = SEQ * BATCH  # 1024 query rows, b-major (row r = b*SEQ + s)
F32 = mybir.dt.float32
BF16 = mybir.dt.bfloat16
AF = mybir.ActivationFunctionType
ALU = mybir.AluOpType
BF = ml_dtypes.bfloat16

_PROGRAM_CACHE: dict = {}


def build_program(ns: int, bg2f: float, reps: int = 1):
    """One SPMD program; per-core data differences come via in_maps.

    reps > 1 statically repeats the whole computation (for wall-clock-delta
    timing of the kernel proper); the output is written identically each rep.
    """
    nchunks = ns // 128
    nc = bacc.Bacc(None, target_bir_lowering=False, debug=False, num_devices=NCORES)

    def inp(nm, shp, dt=F32):
        return nc.declare_dram_parameter(nm, list(shp), dt, isOutput=False)

    qkT_d = inp("qkT", (D, SB), BF16)        # (qb @ Wqk + bias)^T, host-computed
    dkT_d = inp("dkT", (D, ns), BF16)        # datastore-K shard, transposed
    dvp_d = inp("dvp", (128, (ns // 128) * D), BF16)  # V shard, partition-packed
    wvT_d = inp("wvT", (D, D), BF16)         # Wv.T
    wg1T_d = inp("wg1T", (2 * D, D), BF16)   # Wg1.T
    wg2r_d = inp("wg2r", (128, D))           # Wg2 replicated over partitions
    bvr_d = inp("bvr", (128, D))
    bg1r_d = inp("bg1r", (128, D))
    prevN_d = inp("prevN", (128, D))         # prev rows for this core's slice
    prevT_d = inp("prevT", (D, 128), BF16)   # same rows, transposed
    ident_d = inp("ident", (128, 128))
    out_d = nc.declare_dram_parameter("out", [128, D], F32, isOutput=True)

    rg = [list(range(NCORES))]

    def emit_body(nc, tc, pools, rp):
        cp, sp, xp, wgp, mmp, wvp, dp = pools

        def cload(src_ap, shape, tg, dt=F32):
            t = cp.tile(shape, dt, tag=tg, name=rp + tg)
            nc.sync.dma_start(t[:], src_ap)
            return t

        # q-side loads go first in the sync DMA FIFO so the first logits
        # chunk can compute while the bulk dkT load streams in behind it;
        # the first pieces are small so chunk 0 is unblocked early.
        qkT = [cp.tile([128, SB], BF16, tag=f"qkT{k}", name=rp + f"qkT{k}")
               for k in range(4)]
        for k in range(4):
            nc.sync.dma_start(qkT[k][:, 0:512], qkT_d[k * 128:(k + 1) * 128, 0:512])
        dkT = [cp.tile([128, ns], BF16, tag=f"dkT{k}", name=rp + f"dkT{k}")
               for k in range(4)]
        for k in range(4):
            nc.sync.dma_start(dkT[k][:, 0:256], dkT_d[k * 128:(k + 1) * 128, 0:256])
        for k in range(4):
            nc.sync.dma_start(qkT[k][:, 512:SB],
                              qkT_d[k * 128:(k + 1) * 128, 512:SB])
        for o in range(256, ns, 1024):
            o2 = min(o + 1024, ns)
            for k in range(4):
                nc.sync.dma_start(dkT[k][:, o:o2],
                                  dkT_d[k * 128:(k + 1) * 128, o:o2])
        wvT = [cload(wvT_d[k * 128:(k + 1) * 128, :], [128, D], f"wvT{k}", BF16)
               for k in range(4)]
        wg2r = cload(wg2r_d[:], [128, D], "wg2r")
        bvr = cload(bvr_d[:], [128, D], "bvr")
        bg1r = cload(bg1r_d[:], [128, D], "bg1r")
        prevN = cload(prevN_d[:], [128, D], "prevN")
        prevT = [cload(prevT_d[k * 128:(k + 1) * 128, :], [128, 128],
                       f"prevT{k}", BF16) for k in range(4)]
        ident = cload(ident_d[:], [128, 128], "ident")
        ones = cp.tile([128, 1], F32, tag="ones", name=rp + "ones")
        nc.vector.memset(ones[:], 1.0)
        ones_b = cp.tile([128, 1], BF16, tag="ones_b", name=rp + "ones_b")
        nc.vector.memset(ones_b[:], 1.0)
        ident_b = cp.tile([128, 128], BF16, tag="ident_b", name=rp + "ident_b")
        nc.vector.tensor_copy(ident_b[:], ident[:])

        # dv resident in SBUF (bf16, 4 MB): loaded once on the ACT-issued
        # queue in column pieces, reused by both query halves.
        dvt = cp.tile([128, nchunks * D], BF16, tag="dvt", name=rp + "dvt")
        for j in range(0, nchunks, 4):
            nc.scalar.dma_start(dvt[:, j * D:(j + 4) * D],
                                dvp_d[:, j * D:(j + 4) * D])

        # exp(logits) for ALL queries, kept resident (bf16, 8 MB): written by
        # the h=0 pass at full 1024-wide matmuls, re-read by the h=1 AV pass
        # and the sum-exp matmuls.
        e_st = cp.tile([128, nchunks * SB], BF16, tag="e_st", name=rp + "e_st")

        wvacc = [cp.tile([128, SB], BF16, tag=f"wvacc{k}", name=rp + f"wvacc{k}")
                 for k in range(4)]

        cc_in = [dp.tile([SB // 2, 513], BF16, tag=f"ccin{h}",
                         name=rp + f"ccin{h}") for h in range(2)]
        cc_out = [dp.tile([SB // 16, 513], BF16, tag=f"ccout{h}",
                          name=rp + f"ccout{h}") for h in range(2)]

        def sum_exp_both():
            # sum-exp over the datastore shard for BOTH query halves, off the
            # tensor engine: strided chunk-reductions on DVE in 8 pieces per
            # half, interleaved in chunk-readiness order so nearly all of
            # them run during pass 1, then a partition-reduction on GpSimd.
            # Row 0 of each returned broadcast tile is the sum-exp row.
            ev = e_st[:].rearrange("p (j q) -> p j q", j=nchunks)
            npc = nchunks // 8
            accs, spars = [], []
            for h in range(2):
                accs.append(sp.tile([128, 512], F32, tag="scr",
                                    name=rp + f"seacc{h}"))
            for pi in range(8):
                for h in range(2):
                    qs = 512 * h
                    src = ev[:, pi * npc:(pi + 1) * npc, qs:qs + 512] \
                        .transpose([0, 2, 1])
                    if pi == 0:
                        nc.vector.tensor_reduce(accs[h][:], src,
                                                axis=mybir.AxisListType.X,
                                                op=ALU.add)
                    else:
                        ep_t = sp.tile([128, 512], F32, tag="scr",
                                       name=rp + f"sep{h}{pi}")
                        nc.vector.tensor_reduce(ep_t[:], src,
                                                axis=mybir.AxisListType.X,
                                                op=ALU.add)
                        nc.vector.tensor_tensor(accs[h][:], accs[h][:],
                                                ep_t[:], op=ALU.add)
            for h in range(2):
                spar = cp.tile([128, 512], F32, tag=f"spar{h}",
                               name=rp + f"spar{h}")
                nc.gpsimd.partition_all_reduce(spar[:], accs[h][:],
                                               channels=128,
                                               reduce_op=bass_isa.ReduceOp.add)
                spars.append(spar)
            return spars

        def pa_rs(h, spar):
            # unnormalized attn slices + sum-exp column -> cc_in, then RS.
            # pa matmuls first (they don't need the sum-exp, which may still
            # be finishing on DVE/GpSimd), then psc + payload assembly.
            for jj in range(4):
                g = h * 4 + jj
                pa = mmp.tile([128, 512], F32, tag="mm", name=rp + f"pa{g}")
                for k in range(4):
                    nc.tensor.matmul(
                        pa[:], wvacc[k][:, g * 128:(g + 1) * 128],
                        wvT[k][:], start=(k == 0), stop=(k == 3))
                ext = xp.tile([128, 513], BF16, tag="ext", name=rp + f"ext{g}")
                nc.vector.tensor_copy(ext[:, 0:512], pa[:])
                psc = wvp.tile([128, 1], F32, tag="wv", name=rp + f"psc{g}")
                nc.tensor.matmul(psc[:], spar[0:1, jj * 128:(jj + 1) * 128],
                                 ones[0:1, 0:1], start=True, stop=True)
                nc.vector.tensor_copy(ext[:, 512:513], psc[:])
                nc.sync.dma_start(cc_in[h][jj * 128:(jj + 1) * 128, :], ext[:])
            nc.gpsimd.collective_compute(
                "ReduceScatter", ALU.add, replica_groups=rg,
                ins=[cc_in[h].opt()], outs=[cc_out[h].opt()])

        # ---- pass 1: logits for BOTH query halves (paired on the shared dkT
        # stationary), exp into e_st, AV accumulation for half 0.  The next
        # chunk's logits are emitted ahead of this chunk's AV so the tensor
        # queue never stalls on the ACT exp dependency.
        wv_ps = [wvp.tile([128, 512], F32, tag="wv", name=rp + f"wv0{k}")
                 for k in range(4)]

        def logits(j):
            pl0 = mmp.tile([128, 512], F32, tag="mm", name=rp + f"pl{j}a")
            pl1 = mmp.tile([128, 512], F32, tag="mm", name=rp + f"pl{j}b")
            for k in range(4):
                nc.tensor.matmul(
                    pl0[:], dkT[k][:, j * 128:(j + 1) * 128],
                    qkT[k][:, 0:512], start=(k == 0), stop=(k == 3))
                nc.tensor.matmul(
                    pl1[:], dkT[k][:, j * 128:(j + 1) * 128],
                    qkT[k][:, 512:SB], start=(k == 0), stop=(k == 3))
            nc.scalar.activation(e_st[:, j * SB:j * SB + 512], pl0[:], AF.Exp)
            nc.scalar.activation(e_st[:, j * SB + 512:(j + 1) * SB], pl1[:],
                                 AF.Exp)

        logits(0)
        for j in range(nchunks):
            if j + 1 < nchunks:
                logits(j + 1)
            for k in range(4):
                nc.tensor.matmul(
                    wv_ps[k][:],
                    dvt[:, j * D + k * 128:j * D + (k + 1) * 128],
                    e_st[:, j * SB:j * SB + 512],
                    start=(j == 0), stop=(j == nchunks - 1))
        spar0, spar1 = sum_exp_both()
        for k in range(4):
            nc.vector.tensor_copy(wvacc[k][:, 0:512], wv_ps[k][:])
        pa_rs(0, spar0)

        # ---- pass 2: AV for half 1 from stored e (no deps at all)
        wv_ps = [wvp.tile([128, 512], F32, tag="wv", name=rp + f"wv1{k}")
                 for k in range(4)]
        for j in range(nchunks):
            for k in range(4):
                nc.tensor.matmul(
                    wv_ps[k][:],
                    dvt[:, j * D + k * 128:j * D + (k + 1) * 128],
                    e_st[:, j * SB + 512:(j + 1) * SB],
                    start=(j == 0), stop=(j == nchunks - 1))
        for k in range(4):
            nc.vector.tensor_copy(wvacc[k][:, 512:SB], wv_ps[k][:])
        pa_rs(1, spar1)

        # ---- post-RS: this core's 128 query rows.
        # wg1T streams first (no deps -> overlaps the RS wait) and the
        # prev-side half of the gate matmul is accumulated during the wait.
        wg1T = [None] * 8
        for k in list(range(4, 8)) + list(range(4)):
            t = wgp.tile([128, D], BF16, tag="wg", name=rp + f"wg1T{k}")
            nc.sync.dma_start(t[:], wg1T_d[k * 128:(k + 1) * 128, :])
            wg1T[k] = t
        ph2 = mmp.tile([128, D], F32, tag="mm", name=rp + "ph2")
        for k in range(4):
            nc.tensor.matmul(ph2[:], prevT[k][:], wg1T[4 + k][:],
                             start=(k == 0), stop=(k == 3))
        hprev = sp.tile([128, D], F32, tag="scr", name=rp + "hprev")
        nc.vector.tensor_tensor(hprev[:], ph2[:], bg1r[:], op=ALU.add)

        post = cp.tile([128, 513], BF16, tag="post", name=rp + "post")
        nc.sync.dma_start(post[0:64, :], cc_out[0][:])
        nc.sync.dma_start(post[64:128, :], cc_out[1][:])
        a32 = sp.tile([128, D], F32, tag="scr", name=rp + "a32")
        nc.vector.tensor_copy(a32[:], post[:, 0:512])
        s32 = cp.tile([128, 1], F32, tag="s32", name=rp + "s32")
        nc.vector.tensor_copy(s32[:], post[:, 512:513])
        recip = cp.tile([128, 1], F32, tag="recip", name=rp + "recip")
        nc.vector.reciprocal(recip[:], s32[:])
        attn = sp.tile([128, D], F32, tag="scr", name=rp + "attn")
        nc.vector.scalar_tensor_tensor(
            attn[:], a32[:], recip[:], bvr[:],
            op0=ALU.mult, op1=ALU.add)
        attn_b = cp.tile([128, D], BF16, tag="attn_b", name=rp + "attn_b")
        nc.vector.tensor_copy(attn_b[:], attn[:])

        aTall = cp.tile([128, D], BF16, tag="aTall", name=rp + "aTall")
        for k in range(4):
            pt = mmp.tile([128, 128], BF16, tag="mm", name=rp + f"pt{k}")
            nc.tensor.transpose(pt[:], attn_b[:, k * 128:(k + 1) * 128],
                                ident_b[:])
            nc.vector.tensor_copy(aTall[:, k * 128:(k + 1) * 128], pt[:])

        ph = mmp.tile([128, D], F32, tag="mm", name=rp + "ph")
        for k in range(4):
            nc.tensor.matmul(ph[:], aTall[:, k * 128:(k + 1) * 128],
                             wg1T[k][:], start=(k == 0), stop=(k == 3))
        hsum = sp.tile([128, D], F32, tag="scr", name=rp + "hsum")
        nc.vector.tensor_tensor(hsum[:], ph[:], hprev[:], op=ALU.add)
        hrelu = sp.tile([128, D], F32, tag="scr", name=rp + "hrelu")
        nc.scalar.activation(hrelu[:], hsum[:], AF.Relu)

        tmp = sp.tile([128, D], F32, tag="scr", name=rp + "tmp")
        sigp = cp.tile([128, 1], F32, tag="sigp", name=rp + "sigp")
        nc.vector.scalar_tensor_tensor(
            tmp[:], hrelu[:], 1.0, wg2r[:],
            op0=ALU.mult, op1=ALU.mult, accum_out=sigp[:])
        # sigma = 0.5 + 0.5*tanh(0.5*(x + bg2)); tanh shares the Exp table set
        tnh = cp.tile([128, 1], F32, tag="tnh", name=rp + "tnh")
        nc.scalar.activation(tnh[:], sigp[:], AF.Tanh,
                             scale=0.5, bias=0.5 * bg2f)

        dlt = sp.tile([128, D], F32, tag="scr", name=rp + "dlt")
        nc.vector.tensor_tensor(dlt[:], attn[:], prevN[:], op=ALU.subtract)
        x1 = sp.tile([128, D], F32, tag="scr", name=rp + "x1")
        nc.vector.tensor_scalar_mul(x1[:], dlt[:], tnh[:])
        z = sp.tile([128, D], F32, tag="scr", name=rp + "z")
        nc.vector.tensor_tensor(z[:], dlt[:], x1[:], op=ALU.add)
        res = sp.tile([128, D], F32, tag="scr", name=rp + "res")
        nc.vector.scalar_tensor_tensor(
            res[:], z[:], 0.5, prevN[:], op0=ALU.mult, op1=ALU.add)
        nc.sync.dma_start(out_d[:], res[:])

    with tile.TileContext(nc) as tc:
        with (
            tc.tile_pool(name="const", bufs=1) as cp,
            tc.tile_pool(name="scratch", bufs=8) as sp,
            tc.tile_pool(name="xp", bufs=2) as xp,
            tc.tile_pool(name="wgp", bufs=8) as wgp,
            tc.tile_pool(name="mm", bufs=4, space="PSUM") as mmp,
            tc.tile_pool(name="wvp", bufs=4, space="PSUM") as wvp,
            tc.tile_pool(name="dram", bufs=1, space="DRAM") as dp,
        ):
            pools = (cp, sp, xp, wgp, mmp, wvp, dp)
            for rep in range(reps):
                emit_body(nc, tc, pools, f"r{rep}_" if reps > 1 else "")

    nc.finalize()
    return nc


def make_in_maps(q, prev, Wq, bq, Wk, Wv, Wg1, Wg2, bg2, bv, bg1,
                 dstore_k, dstore_v, ns):
    """Host-side sharding + layout prep. Returns per-core input dicts."""
    alpha = (D ** -0.5) / TEMP
    f = np.float32
    qb = np.ascontiguousarray(q.transpose(1, 0, 2).reshape(SB, D), dtype=f)
    prevb = np.ascontiguousarray(prev.transpose(1, 0, 2).reshape(SB, D), dtype=f)
    wqk = (Wq.T.astype(np.float64) @ Wk.astype(np.float64) * alpha).astype(f)
    qkb = ((bq.astype(np.float64) @ Wk.astype(np.float64)) * alpha).astype(f)
    qkT = np.ascontiguousarray((qb @ wqk + qkb).T.astype(BF))  # [D, SB]
    wvT = np.ascontiguousarray(Wv.T.astype(BF))
    wg1T = np.ascontiguousarray(Wg1.T.astype(BF))
    wg2r = np.ascontiguousarray(np.broadcast_to(Wg2.reshape(1, D), (128, D)), dtype=f)
    bvr = np.ascontiguousarray(np.broadcast_to(bv.reshape(1, D), (128, D)), dtype=f)
    bg1r = np.ascontiguousarray(np.broadcast_to(bg1.reshape(1, D), (128, D)), dtype=f)
    ident = np.eye(128, dtype=f)
    half = SB // 2
    sl = half // NCORES  # 64 rows per half per core
    nch = ns // 128

    dkT_all = np.ascontiguousarray(dstore_k.T.astype(BF))      # [D, NTOT]
    dv_bf = dstore_v.astype(BF)                                # [NTOT, D]

    in_maps = []
    for c in range(NCORES):
        rows = np.r_[c * sl:(c + 1) * sl, half + c * sl:half + (c + 1) * sl]
        prevN = np.ascontiguousarray(prevb[rows])
        prevT = np.ascontiguousarray(prevN.T.astype(BF))
        dkT = np.ascontiguousarray(dkT_all[:, c * ns:(c + 1) * ns])
        # partition-major packing: dvp[p, j*D + d] = dv[c*ns + j*128 + p, d]
        dvp = np.ascontiguousarray(
            dv_bf[c * ns:(c + 1) * ns].reshape(nch, 128, D)
            .transpose(1, 0, 2).reshape(128, nch * D))
        in_maps.append({
            "qkT": qkT, "dkT": dkT, "dvp": dvp,
            "wvT": wvT, "wg1T": wg1T, "wg2r": wg2r, "bvr": bvr, "bg1r": bg1r,
            "prevN": prevN, "prevT": prevT, "ident": ident,
        })
    return in_maps


def assemble_output(core_outs):
    """[128,512] per core -> [SEQ, BATCH, D] full output."""
    half = SB // 2
    sl = half // NCORES
    res_bm = np.empty((SB, D), dtype=np.float32)
    for c in range(NCORES):
        res_bm[c * sl:(c + 1) * sl] = core_outs[c][0:sl]
        res_bm[half + c * sl:half + (c + 1) * sl] = core_outs[c][sl:2 * sl]
    return np.ascontiguousarray(
        res_bm.reshape(BATCH, SEQ, D).transpose(1, 0, 2))


def kernel(q, prev_layer_output, Wq, bq, Wk, bk, Wv, bv, Wg1, bg1, Wg2, bg2,
           dstore_k, dstore_v):
    # bk shifts every logit in a row by a constant -> softmax-invariant; unused.
    ns = NTOT // NCORES
    bg2f = float(np.asarray(bg2).reshape(-1)[0])
    key = (ns, bg2f, 1)
    if key not in _PROGRAM_CACHE:
        _PROGRAM_CACHE[key] = build_program(ns, bg2f)
    nc = _PROGRAM_CACHE[key]
    in_maps = make_in_maps(q, prev_layer_output, Wq, bq, Wk, Wv, Wg1, Wg2, bg2,
                           bv, bg1, dstore_k, dstore_v, ns)
    res = run_bass_kernel_spmd(nc, in_maps, list(range(NCORES)))
    return assemble_output([res.results[c]["out"] for c in range(NCORES)])


# revision 38
# speedup vs baseline: 1.1188x; 1.0221x over previous
"""Trainium2 Bass kernel for nn_DatastoreReaderLayer (retrieval kNN attention).

Strategy (8 NeuronCores, datastore sharded over N):
  - Each core owns an N/8 = 4096-row shard of the datastore.
  - K/V weight projections are algebraically absorbed:
      logits = q @ Wq.T @ Wk @ dstore_k.T   (Wqk := Wq.T @ Wk folded on host,
                                             softmax scale folded into Wqk)
      attn   = (softmax @ dstore_v) @ Wv.T  (wv := e @ dstore_v on device)
    so the two big [N,512]x[512,512] projection matmuls vanish.
  - All large matmuls run in bf16 (same 1 cycle/row issue rate as fp32r on
    TRN2, but LDWEIGHTS halves and HBM traffic halves); accumulation is fp32
    in PSUM.  dv is pre-packed on the host into partition-major layout so its
    stream DMAs are long contiguous lines.
  - Softmax without max-subtraction (logits ~ N(0,0.6), exp is safe in fp32);
    partial sum-exp and partial unnormalized attn are combined across cores
    with a single bf16 ReduceScatter per query half (flash-attention merge).
  - Each core finishes the gate MLP for its own 128 query rows and returns a
    [128, 512] slice; the host reassembles the full [256, 4, 512] output.
  - bk is provably a softmax no-op; bq folds into a qk bias vector; bv/bg1/bg2
    are applied exactly on device.
"""

import sys

for _p in ("/opt/trn_rl_repo", "/root/.axon_site/_ro/trn_rl_repo"):
    if _p not in sys.path:
        sys.path.append(_p)

import numpy as np
import ml_dtypes

import concourse.tile as tile
from concourse import bacc, bass_isa, mybir
from concourse.bass_utils import run_bass_kernel_spmd

SEQ, BATCH, D, NTOT = 256, 4, 512, 32768
TEMP = 0.5
NCORES = 8
NS = NTOT // NCORES  # datastore rows per core
SB = SEQ * BATCH  # 1024 query rows, b-major (row r = b*SEQ + s)
F32 = mybir.dt.float32
BF16 = mybir.dt.bfloat16
AF = mybir.ActivationFunctionType
ALU = mybir.AluOpType
BF = ml_dtypes.bfloat16

_PROGRAM_CACHE: dict = {}


def build_program(ns: int, bg2f: float, reps: int = 1):
    """One SPMD program; per-core data differences come via in_maps.

    reps > 1 statically repeats the whole computation (for wall-clock-delta
    timing of the kernel proper); the output is written identically each rep.
    """
    nchunks = ns // 128
    nc = bacc.Bacc(None, target_bir_lowering=False, debug=False, num_devices=NCORES)

    def inp(nm, shp, dt=F32):
        return nc.declare_dram_parameter(nm, list(shp), dt, isOutput=False)

    qkT_d = inp("qkT", (D, SB), BF16)        # (qb @ Wqk + bias)^T, host-computed
    dkT_d = inp("dkT", (D, ns), BF16)        # datastore-K shard, transposed
    dvp_d = inp("dvp", (128, (ns // 128) * D), BF16)  # V shard, partition-packed
    wvT_d = inp("wvT", (D, D), BF16)         # Wv.T
    wg1T_d = inp("wg1T", (2 * D, D), BF16)   # Wg1.T
    wg2r_d = inp("wg2r", (128, D))           # Wg2 replicated over partitions
    bvr_d = inp("bvr", (128, D))
    bg1r_d = inp("bg1r", (128, D))
    prevN_d = inp("prevN", (128, D))         # prev rows for this core's slice
    prevT_d = inp("prevT", (D, 128), BF16)   # same rows, transposed
    ident_d = inp("ident", (128, 128))
    out_d = nc.declare_dram_parameter("out", [128, D], F32, isOutput=True)

    rg = [list(range(NCORES))]

    def emit_body(nc, tc, pools, rp):
        cp, sp, xp, wgp, mmp, wvp, dp = pools

        def cload(src_ap, shape, tg, dt=F32):
            t = cp.tile(shape, dt, tag=tg, name=rp + tg)
            nc.sync.dma_start(t[:], src_ap)
            return t

        # q-side loads go first in the sync DMA FIFO so the first logits
        # chunk can compute while the bulk dkT load streams in behind it;
        # the first pieces are small so chunk 0 is unblocked early.
        qkT = [cp.tile([128, SB], BF16, tag=f"qkT{k}", name=rp + f"qkT{k}")
               for k in range(4)]
        for k in range(4):
            nc.sync.dma_start(qkT[k][:, 0:512], qkT_d[k * 128:(k + 1) * 128, 0:512])
        dkT = [cp.tile([128, ns], BF16, tag=f"dkT{k}", name=rp + f"dkT{k}")
               for k in range(4)]
        for k in range(4):
            nc.sync.dma_start(dkT[k][:, 0:256], dkT_d[k * 128:(k + 1) * 128, 0:256])
        for k in range(4):
            nc.sync.dma_start(qkT[k][:, 512:SB],
                              qkT_d[k * 128:(k + 1) * 128, 512:SB])
        for o in range(256, ns, 1024):
            o2 = min(o + 1024, ns)
            for k in range(4):
                nc.sync.dma_start(dkT[k][:, o:o2],
                                  dkT_d[k * 128:(k + 1) * 128, o:o2])
        wvT = [cload(wvT_d[k * 128:(k + 1) * 128, :], [128, D], f"wvT{k}", BF16)
               for k in range(4)]
        wg2r = cload(wg2r_d[:], [128, D], "wg2r")
        bvr = cload(bvr_d[:], [128, D], "bvr")
        bg1r = cload(bg1r_d[:], [128, D], "bg1r")
        prevN = cload(prevN_d[:], [128, D], "prevN")
        prevT = [cload(prevT_d[k * 128:(k + 1) * 128, :], [128, 128],
                       f"prevT{k}", BF16) for k in range(4)]
        ident = cload(ident_d[:], [128, 128], "ident")
        ones = cp.tile([128, 1], F32, tag="ones", name=rp + "ones")
        nc.vector.memset(ones[:], 1.0)
        ones_b = cp.tile([128, 1], BF16, tag="ones_b", name=rp + "ones_b")
        nc.vector.memset(ones_b[:], 1.0)
        ident_b = cp.tile([128, 128], BF16, tag="ident_b", name=rp + "ident_b")
        nc.vector.tensor_copy(ident_b[:], ident[:])

        # dv resident in SBUF (bf16, 4 MB): loaded once on the ACT-issued
        # queue in column pieces, reused by both query halves.
        dvt = cp.tile([128, nchunks * D], BF16, tag="dvt", name=rp + "dvt")
        for j in range(0, nchunks, 4):
            nc.scalar.dma_start(dvt[:, j * D:(j + 4) * D],
                                dvp_d[:, j * D:(j + 4) * D])

        # exp(logits) for ALL queries, kept resident (bf16, 8 MB): written by
        # the h=0 pass at full 1024-wide matmuls, re-read by the h=1 AV pass
        # and the sum-exp matmuls.
        e_st = cp.tile([128, nchunks * SB], BF16, tag="e_st", name=rp + "e_st")

        wvacc = [cp.tile([128, SB], BF16, tag=f"wvacc{k}", name=rp + f"wvacc{k}")
                 for k in range(4)]

        cc_in = [dp.tile([SB // 2, 513], BF16, tag=f"ccin{h}",
                         name=rp + f"ccin{h}") for h in range(2)]
        cc_out = [dp.tile([SB // 16, 513], BF16, tag=f"ccout{h}",
                          name=rp + f"ccout{h}") for h in range(2)]

        def sum_exp_both():
            # sum-exp over the datastore shard for BOTH query halves, off the
            # tensor engine: strided chunk-reductions on DVE in 8 pieces per
            # half, interleaved in chunk-readiness order so nearly all of
            # them run during pass 1, then a partition-reduction on GpSimd.
            # Row 0 of each returned broadcast tile is the sum-exp row.
            ev = e_st[:].rearrange("p (j q) -> p j q", j=nchunks)
            npc = nchunks // 8
            accs, spars = [], []
            for h in range(2):
                accs.append(sp.tile([128, 512], F32, tag="scr",
                                    name=rp + f"seacc{h}"))
            for pi in range(8):
                for h in range(2):
                    qs = 512 * h
                    src = ev[:, pi * npc:(pi + 1) * npc, qs:qs + 512] \
                        .transpose([0, 2, 1])
                    if pi == 0:
                        nc.vector.tensor_reduce(accs[h][:], src,
                                                axis=mybir.AxisListType.X,
                                                op=ALU.add)
                    else:
                        ep_t = sp.tile([128, 512], F32, tag="scr",
                                       name=rp + f"sep{h}{pi}")
                        nc.vector.tensor_reduce(ep_t[:], src,
                                                axis=mybir.AxisListType.X,
                                                op=ALU.add)
                        nc.vector.tensor_tensor(accs[h][:], accs[h][:],
                                                ep_t[:], op=ALU.add)
            for h in range(2):
                spar = cp.tile([128, 512], F32, tag=f"spar{h}",
                               name=rp + f"spar{h}")
                nc.gpsimd.partition_all_reduce(spar[:], accs[h][:],
                                               channels=128,
                                               reduce_op=bass_isa.ReduceOp.add)
                spars.append(spar)
            return spars

        def pa_rs(h, spar):
            # unnormalized attn slices + sum-exp column -> cc_in, then RS.
            # pa matmuls first (they don't need the sum-exp, which may still
            # be finishing on DVE/GpSimd), then psc + payload assembly.
            for jj in range(4):
                g = h * 4 + jj
                pa = mmp.tile([128, 512], F32, tag="mm", name=rp + f"pa{g}")
                for k in range(4):
                    nc.tensor.matmul(
                        pa[:], wvacc[k][:, g * 128:(g + 1) * 128],
                        wvT[k][:], start=(k == 0), stop=(k == 3))
                ext = xp.tile([128, 513], BF16, tag="ext", name=rp + f"ext{g}")
                nc.vector.tensor_copy(ext[:, 0:512], pa[:])
                psc = wvp.tile([128, 1], F32, tag="wv", name=rp + f"psc{g}")
                nc.tensor.matmul(psc[:], spar[0:1, jj * 128:(jj + 1) * 128],
                                 ones[0:1, 0:1], start=True, stop=True)
                nc.vector.tensor_copy(ext[:, 512:513], psc[:])
                nc.sync.dma_start(cc_in[h][jj * 128:(jj + 1) * 128, :], ext[:])
            nc.gpsimd.collective_compute(
                "ReduceScatter", ALU.add, replica_groups=rg,
                ins=[cc_in[h].opt()], outs=[cc_out[h].opt()])

        # ---- pass 1: logits for BOTH query halves (paired on the shared dkT
        # stationary), exp into e_st, AV accumulation for half 0.  The next
        # chunk's logits are emitted ahead of this chunk's AV so the tensor
        # queue never stalls on the ACT exp dependency.
        wv_ps = [wvp.tile([128, 512], F32, tag="wv", name=rp + f"wv0{k}")
                 for k in range(4)]

        def logits(j):
            pl0 = mmp.tile([128, 512], F32, tag="mm", name=rp + f"pl{j}a")
            pl1 = mmp.tile([128, 512], F32, tag="mm", name=rp + f"pl{j}b")
            for k in range(4):
                nc.tensor.matmul(
                    pl0[:], dkT[k][:, j * 128:(j + 1) * 128],
                    qkT[k][:, 0:512], start=(k == 0), stop=(k == 3))
                nc.tensor.matmul(
                    pl1[:], dkT[k][:, j * 128:(j + 1) * 128],
                    qkT[k][:, 512:SB], start=(k == 0), stop=(k == 3))
            nc.scalar.activation(e_st[:, j * SB:j * SB + 512], pl0[:], AF.Exp)
            nc.scalar.activation(e_st[:, j * SB + 512:(j + 1) * SB], pl1[:],
                                 AF.Exp)

        logits(0)
        for j in range(nchunks):
            if j + 1 < nchunks:
                logits(j + 1)
            for k in range(4):
                nc.tensor.matmul(
                    wv_ps[k][:],
                    dvt[:, j * D + k * 128:j * D + (k + 1) * 128],
                    e_st[:, j * SB:j * SB + 512],
                    start=(j == 0), stop=(j == nchunks - 1))
        spar0, spar1 = sum_exp_both()
        for k in range(4):
            nc.vector.tensor_copy(wvacc[k][:, 0:512], wv_ps[k][:])
        pa_rs(0, spar0)

        # ---- pass 2: AV for half 1 from stored e (no deps at all)
        wv_ps = [wvp.tile([128, 512], F32, tag="wv", name=rp + f"wv1{k}")
                 for k in range(4)]
        for j in range(nchunks):
            for k in range(4):
                nc.tensor.matmul(
                    wv_ps[k][:],
                    dvt[:, j * D + k * 128:j * D + (k + 1) * 128],
                    e_st[:, j * SB + 512:(j + 1) * SB],
                    start=(j == 0), stop=(j == nchunks - 1))
        for k in range(4):
            nc.vector.tensor_copy(wvacc[k][:, 512:SB], wv_ps[k][:])
        pa_rs(1, spar1)

        # ---- post-RS: this core's 128 query rows.
        # wg1T streams first (no deps -> overlaps the RS wait) and the
        # prev-side half of the gate matmul is accumulated during the wait.
        wg1T = [None] * 8
        for k in list(range(4, 8)) + list(range(4)):
            t = wgp.tile([128, D], BF16, tag="wg", name=rp + f"wg1T{k}")
            nc.sync.dma_start(t[:], wg1T_d[k * 128:(k + 1) * 128, :])
            wg1T[k] = t
        ph2 = mmp.tile([128, D], F32, tag="mm", name=rp + "ph2")
        for k in range(4):
            nc.tensor.matmul(ph2[:], prevT[k][:], wg1T[4 + k][:],
                             start=(k == 0), stop=(k == 3))
        hprev = sp.tile([128, D], F32, tag="scr", name=rp + "hprev")
        nc.vector.tensor_tensor(hprev[:], ph2[:], bg1r[:], op=ALU.add)

        post = cp.tile([128, 513], BF16, tag="post", name=rp + "post")
        nc.sync.dma_start(post[0:64, :], cc_out[0][:])
        nc.sync.dma_start(post[64:128, :], cc_out[1][:])
        recip = cp.tile([128, 1], F32, tag="recip", name=rp + "recip")
        nc.vector.reciprocal(recip[:], post[:, 512:513])
        attn = sp.tile([128, D], F32, tag="scr", name=rp + "attn")
        nc.vector.scalar_tensor_tensor(
            attn[:], post[:, 0:512], recip[:], bvr[:],
            op0=ALU.mult, op1=ALU.add)
        attn_b = cp.tile([128, D], BF16, tag="attn_b", name=rp + "attn_b")
        nc.vector.tensor_copy(attn_b[:], attn[:])

        aTall = cp.tile([128, D], BF16, tag="aTall", name=rp + "aTall")
        for k in range(4):
            pt = mmp.tile([128, 128], BF16, tag="mm", name=rp + f"pt{k}")
            nc.tensor.transpose(pt[:], attn_b[:, k * 128:(k + 1) * 128],
                                ident_b[:])
            nc.vector.tensor_copy(aTall[:, k * 128:(k + 1) * 128], pt[:])

        ph = mmp.tile([128, D], F32, tag="mm", name=rp + "ph")
        for k in range(4):
            nc.tensor.matmul(ph[:], aTall[:, k * 128:(k + 1) * 128],
                             wg1T[k][:], start=(k == 0), stop=(k == 3))
        hsum = sp.tile([128, D], F32, tag="scr", name=rp + "hsum")
        nc.vector.tensor_tensor(hsum[:], ph[:], hprev[:], op=ALU.add)
        hrelu = sp.tile([128, D], F32, tag="scr", name=rp + "hrelu")
        nc.scalar.activation(hrelu[:], hsum[:], AF.Relu)

        tmp = sp.tile([128, D], F32, tag="scr", name=rp + "tmp")
        sigp = cp.tile([128, 1], F32, tag="sigp", name=rp + "sigp")
        nc.vector.scalar_tensor_tensor(
            tmp[:], hrelu[:], 1.0, wg2r[:],
            op0=ALU.mult, op1=ALU.mult, accum_out=sigp[:])
        # sigma = 0.5 + 0.5*tanh(0.5*(x + bg2)); tanh shares the Exp table set
        tnh = cp.tile([128, 1], F32, tag="tnh", name=rp + "tnh")
        nc.scalar.activation(tnh[:], sigp[:], AF.Tanh,
                             scale=0.5, bias=0.5 * bg2f)

        dlt = sp.tile([128, D], F32, tag="scr", name=rp + "dlt")
        nc.vector.tensor_tensor(dlt[:], attn[:], prevN[:], op=ALU.subtract)
        x1 = sp.tile([128, D], F32, tag="scr", name=rp + "x1")
        nc.vector.tensor_scalar_mul(x1[:], dlt[:], tnh[:])
        z = sp.tile([128, D], F32, tag="scr", name=rp + "z")
        nc.vector.tensor_tensor(z[:], dlt[:], x1[:], op=ALU.add)
        res = sp.tile([128, D], F32, tag="scr", name=rp + "res")
        nc.vector.scalar_tensor_tensor(
            res[:], z[:], 0.5, prevN[:], op0=ALU.mult, op1=ALU.add)
        nc.sync.dma_start(out_d[:], res[:])

    with tile.TileContext(nc) as tc:
        with (
            tc.tile_pool(name="const", bufs=1) as cp,
            tc.tile_pool(name="scratch", bufs=8) as sp,
            tc.tile_pool(name="xp", bufs=2) as xp,
            tc.tile_pool(name="wgp", bufs=8) as wgp,
            tc.tile_pool(name="mm", bufs=4, space="PSUM") as mmp,
            tc.tile_pool(name="wvp", bufs=4, space="PSUM") as wvp,
            tc.tile_pool(name="dram", bufs=1, space="DRAM") as dp,
        ):
            pools = (cp, sp, xp, wgp, mmp, wvp, dp)
            for rep in range(reps):
                emit_body(nc, tc, pools, f"r{rep}_" if reps > 1 else "")

    nc.finalize()
    return nc


def make_in_maps(q, prev, Wq, bq, Wk, Wv, Wg1, Wg2, bg2, bv, bg1,
                 dstore_k, dstore_v, ns):
    """Host-side sharding + layout prep. Returns per-core input dicts."""
    alpha = (D ** -0.5) / TEMP
    f = np.float32
    qb = np.ascontiguousarray(q.transpose(1, 0, 2).reshape(SB, D), dtype=f)
    prevb = np.ascontiguousarray(prev.transpose(1, 0, 2).reshape(SB, D), dtype=f)
    wqk = (Wq.T.astype(np.float64) @ Wk.astype(np.float64) * alpha).astype(f)
    qkb = ((bq.astype(np.float64) @ Wk.astype(np.float64)) * alpha).astype(f)
    qkT = np.ascontiguousarray((qb @ wqk + qkb).T.astype(BF))  # [D, SB]
    wvT = np.ascontiguousarray(Wv.T.astype(BF))
    wg1T = np.ascontiguousarray(Wg1.T.astype(BF))
    wg2r = np.ascontiguousarray(np.broadcast_to(Wg2.reshape(1, D), (128, D)), dtype=f)
    bvr = np.ascontiguousarray(np.broadcast_to(bv.reshape(1, D), (128, D)), dtype=f)
    bg1r = np.ascontiguousarray(np.broadcast_to(bg1.reshape(1, D), (128, D)), dtype=f)
    ident = np.eye(128, dtype=f)
    half = SB // 2
    sl = half // NCORES  # 64 rows per half per core
    nch = ns // 128

    dkT_all = np.ascontiguousarray(dstore_k.T.astype(BF))      # [D, NTOT]
    dv_bf = dstore_v.astype(BF)                                # [NTOT, D]

    in_maps = []
    for c in range(NCORES):
        rows = np.r_[c * sl:(c + 1) * sl, half + c * sl:half + (c + 1) * sl]
        prevN = np.ascontiguousarray(prevb[rows])
        prevT = np.ascontiguousarray(prevN.T.astype(BF))
        dkT = np.ascontiguousarray(dkT_all[:, c * ns:(c + 1) * ns])
        # partition-major packing: dvp[p, j*D + d] = dv[c*ns + j*128 + p, d]
        dvp = np.ascontiguousarray(
            dv_bf[c * ns:(c + 1) * ns].reshape(nch, 128, D)
            .transpose(1, 0, 2).reshape(128, nch * D))
        in_maps.append({
            "qkT": qkT, "dkT": dkT, "dvp": dvp,
            "wvT": wvT, "wg1T": wg1T, "wg2r": wg2r, "bvr": bvr, "bg1r": bg1r,
            "prevN": prevN, "prevT": prevT, "ident": ident,
        })
    return in_maps


def assemble_output(core_outs):
    """[128,512] per core -> [SEQ, BATCH, D] full output."""
    half = SB // 2
    sl = half // NCORES
    res_bm = np.empty((SB, D), dtype=np.float32)
    for c in range(NCORES):
        res_bm[c * sl:(c + 1) * sl] = core_outs[c][0:sl]
        res_bm[half + c * sl:half + (c + 1) * sl] = core_outs[c][sl:2 * sl]
    return np.ascontiguousarray(
        res_bm.reshape(BATCH, SEQ, D).transpose(1, 0, 2))


def kernel(q, prev_layer_output, Wq, bq, Wk, bk, Wv, bv, Wg1, bg1, Wg2, bg2,
           dstore_k, dstore_v):
    # bk shifts every logit in a row by a constant -> softmax-invariant; unused.
    ns = NTOT // NCORES
    bg2f = float(np.asarray(bg2).reshape(-1)[0])
    key = (ns, bg2f, 1)
    if key not in _PROGRAM_CACHE:
        _PROGRAM_CACHE[key] = build_program(ns, bg2f)
    nc = _PROGRAM_CACHE[key]
    in_maps = make_in_maps(q, prev_layer_output, Wq, bq, Wk, Wv, Wg1, Wg2, bg2,
                           bv, bg1, dstore_k, dstore_v, ns)
    res = run_bass_kernel_spmd(nc, in_maps, list(range(NCORES)))
    return assemble_output([res.results[c]["out"] for c in range(NCORES)])
